# revision 3
# baseline (speedup 1.0000x reference)
"""Trainium2 Bass kernel for nn_GCNCountry (gnn_message_passing).

Reference computation:
    h  = leaky_relu(adj @ (x @ W_gc) + b_gc)        [8192, 1024]
    h  = leaky_relu(h @ W1 + b1)                    [8192, 512]
    h  = dropout(h, p=0.3)  (deterministic mask from drop_u)
    out = (h @ W2 + b2)[0]                          [1]

Only row 0 of the final output is returned, so the whole computation
collapses to the row-0 slice:
    v   = adj[0] @ x                                [512]   (8192-long contraction)
    h1  = leaky_relu(v @ W_gc + b_gc)               [1024]
    h2  = leaky_relu(h1 @ W1 + b1)                  [512]
    out = (mask * h2) @ W2 + b2                     [1]

Device strategy (8 NeuronCores):
  Launch A: contraction over nodes row-sharded 1024 rows/core
            (per the row-parallel sharding hint); each core emits a
            partial v [512]; host sums the 8 partials.
  Launch B: MLP layer 1 column-sharded (128 cols of W_gc per core) and
            layer 2 row-sharded (matching 128 rows of W1 per core);
            each core emits a partial of (h1 @ W1) [512]; host sums,
            then applies the tiny 512-element epilogue (bias, leaky,
            dropout mask, 512-long dot with W2).
"""

import numpy as np

import concourse.mybir as mybir
from concourse import bacc
from concourse.tile import TileContext
from concourse.bass_utils import run_bass_kernel_spmd


def _new_nc():
    return bacc.Bacc("TRN2", target_bir_lowering=False, debug=False,
                     num_devices=N_CORES)

F32 = mybir.dt.float32
N_CORES = 8
N_NODES, N_FEAT, N_HID1, N_HID2 = 8192, 512, 1024, 512
ROWS_PER_CORE = N_NODES // N_CORES          # 1024
KT1 = ROWS_PER_CORE // 128                  # 8 contraction tiles (phase 1)
H1_PER_CORE = N_HID1 // N_CORES             # 128
QT2 = N_FEAT // 128                         # 4 contraction tiles (phase 2 layer 1)
SLOPE = 0.01
DROP_P = 0.3

_CACHE = {}


def _build_phase1():
    """Per core: vp[1, 512] = a_col.T-contraction over 1024 rows of x.

    a  [128, KT1]      adj[0] slice, column-major chunks: a[p, k] = adj0[k*128+p]
    xs [1024, 512]     x row-slice (natural layout)
    """
    nc = _new_nc()
    a = nc.dram_tensor("a", [128, KT1], F32, kind="ExternalInput")
    xs = nc.dram_tensor("xs", [ROWS_PER_CORE, N_FEAT], F32, kind="ExternalInput")
    vp = nc.dram_tensor("vp", [1, N_FEAT], F32, kind="ExternalOutput")

    with TileContext(nc) as tc:
        with (
            tc.tile_pool(name="adj", bufs=1) as apool,
            tc.tile_pool(name="xtiles", bufs=KT1) as xpool,
            tc.tile_pool(name="out", bufs=1) as opool,
            tc.tile_pool(name="psum", bufs=1, space="PSUM") as ppool,
        ):
            a_t = apool.tile([128, KT1], F32)
            nc.sync.dma_start(a_t[:], a[:])
            ps = ppool.tile([1, N_FEAT], F32)
            for k in range(KT1):
                xt = xpool.tile([128, N_FEAT], F32, tag="x")
                nc.sync.dma_start(xt[:], xs[k * 128:(k + 1) * 128, :])
                nc.tensor.matmul(
                    ps[:], a_t[:, k:k + 1], xt[:],
                    start=(k == 0), stop=(k == KT1 - 1),
                )
            ot = opool.tile([1, N_FEAT], F32)
            nc.vector.tensor_copy(ot[:], ps[:])
            nc.sync.dma_start(vp[:], ot[:])
    nc.compile()
    return nc


def _build_phase2():
    """Per core: p2[1, 512] = leaky(v @ Wgc_c + bgc_c) @ W1_c   (partial over
    this core's 128 hid1 units).

    vc [128, QT2]      v column-form: vc[p, q] = v[q*128+p]
    wg [128, QT2*128]  W_gc slice: wg[p, q*128+m] = W_gc[q*128+p, c*128+m]
    bg [128, 1]        b_gc[c*128:(c+1)*128]
    w1 [128, 512]      W1[c*128:(c+1)*128, :]  (natural layout)
    """
    nc = _new_nc()
    vc = nc.dram_tensor("vc", [128, QT2], F32, kind="ExternalInput")
    wg = nc.dram_tensor("wg", [128, QT2 * 128], F32, kind="ExternalInput")
    bg = nc.dram_tensor("bg", [128, 1], F32, kind="ExternalInput")
    w1 = nc.dram_tensor("w1", [128, N_HID2], F32, kind="ExternalInput")
    p2 = nc.dram_tensor("p2", [1, N_HID2], F32, kind="ExternalOutput")

    with TileContext(nc) as tc:
        with (
            tc.tile_pool(name="sbuf", bufs=1) as pool,
            tc.tile_pool(name="psum", bufs=2, space="PSUM") as ppool,
        ):
            vc_t = pool.tile([128, QT2], F32, tag="vc")
            wg_t = pool.tile([128, QT2 * 128], F32, tag="wg")
            bg_t = pool.tile([128, 1], F32, tag="bg")
            w1_t = pool.tile([128, N_HID2], F32, tag="w1")
            nc.sync.dma_start(vc_t[:], vc[:])
            nc.sync.dma_start(wg_t[:], wg[:])
            nc.sync.dma_start(bg_t[:], bg[:])
            nc.sync.dma_start(w1_t[:], w1[:])

            # layer 1, column form: ps1[128, 1] = Wgc_c.T @ v
            ps1 = ppool.tile([128, 1], F32, tag="ps1")
            for q in range(QT2):
                nc.tensor.matmul(
                    ps1[:], wg_t[:, q * 128:(q + 1) * 128], vc_t[:, q:q + 1],
                    start=(q == 0), stop=(q == QT2 - 1),
                )
            # h1 = leaky(ps1 + bg)
            t1 = pool.tile([128, 1], F32, tag="t1")
            nc.vector.tensor_scalar_add(t1[:], ps1[:], bg_t[:, 0:1])
            h1 = pool.tile([128, 1], F32, tag="h1")
            nc.vector.scalar_tensor_tensor(
                h1[:], t1[:], SLOPE, t1[:],
                op0=mybir.AluOpType.mult, op1=mybir.AluOpType.max,
            )
            # layer 2 partial: ps2[1, 512] = h1.T @ W1_c
            ps2 = ppool.tile([1, N_HID2], F32, tag="ps2")
            nc.tensor.matmul(ps2[:], h1[:, 0:1], w1_t[:], start=True, stop=True)
            ot = pool.tile([1, N_HID2], F32, tag="out")
            nc.vector.tensor_copy(ot[:], ps2[:])
            nc.sync.dma_start(p2[:], ot[:])
    nc.compile()
    return nc


def _get(name, builder):
    if name not in _CACHE:
        _CACHE[name] = builder()
    return _CACHE[name]


_LAST_RESULTS = {}


def _run(name, builder, in_maps, **kw):
    nc = _get(name, builder)
    res = run_bass_kernel_spmd(nc, in_maps, core_ids=list(range(N_CORES)), **kw)
    _LAST_RESULTS[name] = res
    return res.results


def kernel(**inputs):
    f = lambda k: np.ascontiguousarray(np.asarray(inputs[k]), dtype=np.float32)
    x, adj = f("x"), np.asarray(inputs["adj"])
    W_gc, b_gc = f("W_gc"), f("b_gc")
    W1, b1 = f("W1"), f("b1")
    W2, b2 = f("W2"), f("b2")
    drop_u = np.asarray(inputs["drop_u"])

    adj0 = np.ascontiguousarray(np.asarray(adj[0]), dtype=np.float32)  # [8192]
    drop0 = np.asarray(drop_u[0])                                      # [512]

    # ---- Launch A: v = adj[0] @ x, row-sharded over nodes ----
    in_maps1 = []
    for c in range(N_CORES):
        sl = slice(c * ROWS_PER_CORE, (c + 1) * ROWS_PER_CORE)
        a_c = np.ascontiguousarray(adj0[sl].reshape(KT1, 128).T)       # [128, KT1]
        in_maps1.append({"a": a_c, "xs": np.ascontiguousarray(x[sl])})
    res1 = _run("p1", _build_phase1, in_maps1)
    v = np.stack([r["vp"][0] for r in res1]).sum(axis=0, dtype=np.float32)  # [512]

    # ---- Launch B: p = leaky(v@W_gc+b_gc) @ W1, sharded over hid1 ----
    vc = np.ascontiguousarray(v.reshape(QT2, 128).T)                   # [128, QT2]
    in_maps2 = []
    for c in range(N_CORES):
        sl = slice(c * H1_PER_CORE, (c + 1) * H1_PER_CORE)
        wg_c = np.ascontiguousarray(
            W_gc[:, sl].reshape(QT2, 128, H1_PER_CORE)
            .transpose(1, 0, 2).reshape(128, QT2 * H1_PER_CORE))
        in_maps2.append({
            "vc": vc,
            "wg": wg_c,
            "bg": np.ascontiguousarray(b_gc[sl].reshape(128, 1)),
            "w1": np.ascontiguousarray(W1[sl, :]),
        })
    res2 = _run("p2", _build_phase2, in_maps2)
    p = np.stack([r["p2"][0] for r in res2]).sum(axis=0, dtype=np.float32)  # [512]

    # ---- Host epilogue: 512-element bias+leaky+mask, 512-long dot ----
    h2 = p + b1
    h2 = np.where(h2 >= 0, h2, np.float32(SLOPE) * h2).astype(np.float32)
    h2d = np.where(drop0 >= np.float32(DROP_P),
                   h2 / np.float32(1.0 - DROP_P), np.float32(0)).astype(np.float32)
    out = (h2d @ W2 + b2).astype(np.float32)                           # [1]
    return out


# revision 6
# speedup vs baseline: 1.2470x; 1.2470x over previous
"""Trainium2 Bass kernel for nn_GCNCountry (gnn_message_passing).

Reference computation:
    h  = leaky_relu(adj @ (x @ W_gc) + b_gc)        [8192, 1024]
    h  = leaky_relu(h @ W1 + b1)                    [8192, 512]
    h  = dropout(h, p=0.3)  (deterministic mask from drop_u)
    out = (h @ W2 + b2)[0]                          [1]

Only row 0 of the final output is returned, so the computation collapses
to the row-0 slice:
    v   = adj[0] @ x                                [512]   (8192-long contraction)
    h1  = leaky_relu(v @ W_gc + b_gc)               [1024]
    h2  = leaky_relu(h1 @ W1 + b1)                  [512]
    out = (mask * h2) @ W2 + b2                     [1]

Device strategy (8 NeuronCores, bf16 inputs / f32 accumulation):
  Launch A: contraction over nodes row-sharded 1024 rows/core (per the
            row-parallel sharding hint); each core emits a partial
            v [512] in f32; host sums the 8 partials.
  Launch B: MLP layer 1 column-sharded (128 cols of W_gc per core, bias
            folded into the matmul accumulation) and layer 2 row-sharded
            (matching 128 rows of W1); each core emits an f32 partial of
            (h1 @ W1) [512]; host sums, then applies the tiny
            512-element epilogue (bias, leaky, dropout mask, dot W2).
"""

import numpy as np
import ml_dtypes

import concourse.mybir as mybir
from concourse import bacc
from concourse.tile import TileContext
from concourse.bass_utils import run_bass_kernel_spmd

F32 = mybir.dt.float32
BF16 = mybir.dt.bfloat16
NP_BF16 = ml_dtypes.bfloat16

N_CORES = 8
N_NODES, N_FEAT, N_HID1, N_HID2 = 8192, 512, 1024, 512
ROWS_PER_CORE = N_NODES // N_CORES          # 1024
KT1 = ROWS_PER_CORE // 128                  # 8 contraction tiles (phase 1)
H1_PER_CORE = N_HID1 // N_CORES             # 128
QT2 = N_FEAT // 128                         # 4 contraction tiles (phase 2 layer 1)
SLOPE = 0.01
DROP_P = 0.3

# phase-2 packed layout: [vc | wg | w1] along the free dim
P2_VC0 = 0
P2_WG0 = QT2                                # 4
P2_W10 = QT2 + QT2 * 128                    # 516
P2_W = QT2 + QT2 * 128 + N_HID2             # 1028

_CACHE = {}


def _new_nc():
    return bacc.Bacc("TRN2", target_bir_lowering=False, debug=False,
                     num_devices=N_CORES)


def _build_phase1():
    """Per core: vp[1, 512] (f32) = sum_k a_k.T @ x_k over this core's
    1024 node rows.

    xa [1024, 513] bf16, row r = k*128+p:
        xa[r, 0]   = adj0[core_off + r]     (the lhsT column)
        xa[r, 1:]  = x[core_off + r, :]
    """
    nc = _new_nc()
    xa = nc.dram_tensor("xa", [ROWS_PER_CORE, 1 + N_FEAT], BF16,
                        kind="ExternalInput")
    vp = nc.dram_tensor("vp", [1, N_FEAT], F32, kind="ExternalOutput")

    with TileContext(nc) as tc:
        with (
            tc.tile_pool(name="xtiles", bufs=KT1) as xpool,
            tc.tile_pool(name="out", bufs=1) as opool,
            tc.tile_pool(name="psum", bufs=1, space="PSUM") as ppool,
        ):
            ps = ppool.tile([1, N_FEAT], F32)
            for k in range(KT1):
                xt = xpool.tile([128, 1 + N_FEAT], BF16, tag="x")
                nc.sync.dma_start(xt[:], xa[k * 128:(k + 1) * 128, :])
                nc.tensor.matmul(
                    ps[:], xt[:, 0:1], xt[:, 1:1 + N_FEAT],
                    start=(k == 0), stop=(k == KT1 - 1),
                )
            ot = opool.tile([1, N_FEAT], F32)
            nc.vector.tensor_copy(ot[:], ps[:])
            nc.sync.dma_start(vp[:], ot[:])
    nc.compile()
    return nc


def _build_phase2():
    """Per core: p2[1, 512] (f32) = leaky(Wgc_c.T @ v + bgc_c).T @ W1_c,
    the partial of h1 @ W1 contributed by this core's 128 hid1 units.

    wv [128, 1028] bf16 packed columns:
        [:, 0:4]      vc: v column-form, vc[p, q] = v[q*128+p]
        [:, 4:516]    wg: wg[p, q*128+m] = W_gc[q*128+p, c*128+m]
        [:, 516:1028] w1: W1[c*128:(c+1)*128, :]
    bgr [1, 128] bf16: b_gc[c*128:(c+1)*128] (bias row, folded into the
        matmul accumulation against a constant-1 rhs)
    """
    nc = _new_nc()
    wv = nc.dram_tensor("wv", [128, P2_W], BF16, kind="ExternalInput")
    bgr = nc.dram_tensor("bgr", [1, 128], BF16, kind="ExternalInput")
    p2 = nc.dram_tensor("p2", [1, N_HID2], F32, kind="ExternalOutput")

    one_bf16 = nc.const_aps.aps[(BF16, 1.0)]

    with TileContext(nc) as tc:
        with (
            tc.tile_pool(name="sbuf", bufs=1) as pool,
            tc.tile_pool(name="psum", bufs=2, space="PSUM") as ppool,
        ):
            wv_t = pool.tile([128, P2_W], BF16, tag="wv")
            bg_t = pool.tile([1, 128], BF16, tag="bg")
            nc.sync.dma_start(wv_t[:], wv[:])
            nc.sync.dma_start(bg_t[:], bgr[:])

            # layer 1 column form: ps1[128, 1] = Wgc_c.T @ v + bgc_c
            ps1 = ppool.tile([128, 1], F32, tag="ps1")
            for q in range(QT2):
                nc.tensor.matmul(
                    ps1[:],
                    wv_t[:, P2_WG0 + q * 128:P2_WG0 + (q + 1) * 128],
                    wv_t[:, P2_VC0 + q:P2_VC0 + q + 1],
                    start=(q == 0), stop=False,
                )
            # bias via K=1 matmul against constant 1.0
            nc.tensor.matmul(ps1[:], bg_t[0:1, :], one_bf16[0:1, 0:1],
                             start=False, stop=True)
            # h1 = leaky(ps1) (copy out of PSUM first: both DVE tensor
            # operands cannot live in PSUM), cast bf16 for layer 2
            h1f = pool.tile([128, 1], F32, tag="h1f")
            nc.vector.tensor_copy(h1f[:], ps1[:])
            h1 = pool.tile([128, 1], BF16, tag="h1")
            nc.vector.scalar_tensor_tensor(
                h1[:], h1f[:], SLOPE, h1f[:],
                op0=mybir.AluOpType.mult, op1=mybir.AluOpType.max,
            )
            # layer 2 partial: ps2[1, 512] = h1.T @ W1_c
            ps2 = ppool.tile([1, N_HID2], F32, tag="ps2")
            nc.tensor.matmul(ps2[:], h1[:, 0:1], wv_t[:, P2_W10:P2_W10 + N_HID2],
                             start=True, stop=True)
            ot = pool.tile([1, N_HID2], F32, tag="out")
            nc.vector.tensor_copy(ot[:], ps2[:])
            nc.sync.dma_start(p2[:], ot[:])
    nc.compile()
    return nc


def _get(name, builder):
    if name not in _CACHE:
        _CACHE[name] = builder()
    return _CACHE[name]


_LAST_RESULTS = {}


def _run(name, builder, in_maps, **kw):
    nc = _get(name, builder)
    res = run_bass_kernel_spmd(nc, in_maps, core_ids=list(range(N_CORES)), **kw)
    _LAST_RESULTS[name] = res
    return res.results


def kernel(**inputs):
    f = lambda k: np.ascontiguousarray(np.asarray(inputs[k]), dtype=np.float32)
    x = f("x")
    adj0 = np.ascontiguousarray(np.asarray(inputs["adj"][0]), dtype=np.float32)
    W_gc, b_gc = f("W_gc"), f("b_gc")
    W1, b1 = f("W1"), f("b1")
    W2, b2 = f("W2"), f("b2")
    drop0 = np.asarray(inputs["drop_u"][0])

    # ---- Launch A: v = adj[0] @ x, row-sharded over nodes ----
    x_b = x.astype(NP_BF16)
    a_b = adj0.astype(NP_BF16)
    in_maps1 = []
    for c in range(N_CORES):
        sl = slice(c * ROWS_PER_CORE, (c + 1) * ROWS_PER_CORE)
        xa = np.empty((ROWS_PER_CORE, 1 + N_FEAT), NP_BF16)
        xa[:, 0] = a_b[sl]
        xa[:, 1:] = x_b[sl]
        in_maps1.append({"xa": xa})
    res1 = _run("p1", _build_phase1, in_maps1)
    v = np.stack([r["vp"][0] for r in res1]).sum(axis=0, dtype=np.float32)  # [512]

    # ---- Launch B: p = (leaky(v@W_gc+b_gc) @ W1) partials over hid1 ----
    vc = np.ascontiguousarray(v.astype(NP_BF16).reshape(QT2, 128).T)
    Wgc_b = W_gc.astype(NP_BF16)
    W1_b = W1.astype(NP_BF16)
    bgc_b = b_gc.astype(NP_BF16)
    in_maps2 = []
    for c in range(N_CORES):
        sl = slice(c * H1_PER_CORE, (c + 1) * H1_PER_CORE)
        wv = np.empty((128, P2_W), NP_BF16)
        wv[:, P2_VC0:P2_VC0 + QT2] = vc
        wv[:, P2_WG0:P2_WG0 + QT2 * 128] = (
            Wgc_b[:, sl].reshape(QT2, 128, H1_PER_CORE)
            .transpose(1, 0, 2).reshape(128, QT2 * H1_PER_CORE))
        wv[:, P2_W10:P2_W10 + N_HID2] = W1_b[sl, :]
        in_maps2.append({"wv": wv, "bgr": bgc_b[sl].reshape(1, 128)})
    res2 = _run("p2", _build_phase2, in_maps2)
    p = np.stack([r["p2"][0] for r in res2]).sum(axis=0, dtype=np.float32)  # [512]

    # ---- Host epilogue: 512-element bias+leaky+mask, 512-long dot ----
    h2 = p + b1
    h2 = np.where(h2 >= 0, h2, np.float32(SLOPE) * h2).astype(np.float32)
    h2d = np.where(drop0 >= np.float32(DROP_P),
                   h2 / np.float32(1.0 - DROP_P), np.float32(0)).astype(np.float32)
    out = (h2d @ W2 + b2).astype(np.float32)                           # [1]
    return out
